# revision 1
# baseline (speedup 1.0000x reference)
"""GroupNorm + per-frame spatial attention block on 8 TRN2 NeuronCores.

Problem shape: x (1, 512, 4, 64, 64) f32.
  y   = GroupNorm32(x) (stats over (c/32, t, h, w) -> global over all frames)
  tok = y as (t, hw=4096, c=512)
  q,k,v = tok @ w{q,k,v}.T + b ; per-frame softmax(q k^T / sqrt(c)) v
  out = attn @ wp.T + bp ; return x + out

Sharding: core i handles frame f=i//2, query-half h=i%2 (2048 queries).
Each core redundantly computes K/V for its whole frame (cheaper than an
intra-pair all-gather).

Two launches (a fleet-wide collective barrier costs ~65us of latency, so
the tiny GroupNorm stats reduction is done as its own collective-free
kernel; the host combines the 8x[128,8] partial sums while "gathering"):
  kernel 1: per-core partial sum/sumsq over its disjoint half-frame.
  host:     combine partials -> per-channel scale/bias (512 numbers).
  kernel 2: normalize + qkv + attention + proj + residual.

Math simplifications used (exact, not approximations):
  - bk drops out of softmax (adds a per-query constant to scores).
  - bv passes through attention unchanged (softmax weights sum to 1), so
    it is folded into the proj bias on the host: bp_eff = bp + wp @ bv.
  - softmax computed without max-subtraction: scores ~ N(0,1) after the
    1/sqrt(c) scaling, exp() is safe in f32.

Device layouts (per core):
  xf   [512, 4096] f32 : frame, columns rolled so the local half is first
  w*T  [512, 512] bf16 : transposed weights [c_in, c_out] (contraction on
                         partitions)
  Scores are computed transposed, sT[kt,qt] = k_cm^T q_cm, so the exp'd
  probabilities feed the PV matmul (channel-major out) with zero on-chip
  transposes.  The softmax denominator is accumulated on the vector
  engine (f32), partition-reduced with a ones-matmul, reciprocal'd, and
  rank-1-broadcast on the PE; since proj is linear, normalization is
  applied after the proj matmul so the PV psum banks free up immediately.

Measured on 8xTRN2 (NTFF profile): ~27.5us (stats) + ~392us (main)
~= 420us total; main kernel TensorE-active ~331us (~80% occupancy,
~90% of bf16 stream peak while active).
"""

import numpy as np
import ml_dtypes

import concourse.bass as bass
import concourse.bacc as bacc
import concourse.tile as tile
from concourse import mybir
from concourse.bass_utils import run_bass_kernel_spmd

C = 512
T = 4
HW = 64 * 64          # tokens per frame
HALF = HW // 2        # local queries per core
G = 32                # groups
N_CORES = 8
EPS = 1e-6
NG_ELEMS = (C // G) * T * HW   # elements per group in the full tensor
CB = C // 128         # 4 channel blocks
QG = HALF // 512      # 4 query groups of 512
NKT = HW // 128       # 32 key chunks of 128
SCALE = float(C) ** -0.5

BF16 = mybir.dt.bfloat16
F32 = mybir.dt.float32
AX = mybir.AxisListType
AF = mybir.ActivationFunctionType
OP = mybir.AluOpType

_CACHE = {}


# ---------------------------------------------------------------- kernel 1
def _build_stats():
    nc = bacc.Bacc("TRN2", target_bir_lowering=False, debug=False,
                   num_devices=N_CORES)
    xh = nc.declare_dram_parameter("xh", [C, HALF], F32, isOutput=False)
    pstats = nc.declare_dram_parameter("pstats", [128, 2 * CB], F32,
                                       isOutput=True)
    with tile.TileContext(nc) as tc:
        with tc.tile_pool(name="xt", bufs=CB) as xt_pool, \
             tc.tile_pool(name="scr", bufs=2) as scr_pool, \
             tc.tile_pool(name="st", bufs=1) as st_pool:
            # sums on DVE, sums-of-squares on ACT: the two run in parallel
            stats_sb = st_pool.tile([128, 2 * CB], F32, name="stats")
            stats2_sb = st_pool.tile([128, CB], F32, name="stats2")
            for j in range(CB):
                xt = xt_pool.tile([128, HALF], F32, tag="xt", name="xt")
                eng = nc.sync if j % 2 == 0 else nc.scalar
                eng.dma_start(xt[:, :], xh[j * 128:(j + 1) * 128, :])
                nc.vector.reduce_sum(stats_sb[:, j:j + 1], xt[:, :], axis=AX.X)
                scr = scr_pool.tile([128, HALF], F32, tag="scr", name="scr")
                nc.scalar.activation(scr[:, :], xt[:, :], AF.Square,
                                     accum_out=stats2_sb[:, j:j + 1])
            nc.vector.tensor_copy(stats_sb[:, CB:2 * CB], stats2_sb[:, :])
            nc.sync.dma_start(pstats[:, :], stats_sb[:, :])
    nc.finalize()
    return nc


# ---------------------------------------------------------------- kernel 2
def _body(tc, P):
    from contextlib import ExitStack

    nc = tc.nc
    with ExitStack() as ctx:
        consts = ctx.enter_context(tc.tile_pool(name="consts", bufs=1))

        def load_const(name, shape, dtype, src, engine=None):
            t_ = consts.tile(shape, dtype, name=name)
            (engine or nc.scalar).dma_start(t_[:, :], src)
            return t_

        # The xn chain is the critical path: scale/bias ride the scalar
        # HWDGE ring, the 8MB xf load rides the sync ring in half-tiles
        # (the two rings share the 16 SDMA engines, so splitting xf across
        # them measured neutral), and the weights follow on the scalar ring.
        scl_sb = load_const("scl", [128, CB], F32, P["scl2d"][:, :])
        bia_sb = load_const("bia", [128, CB], F32, P["bia2d"][:, :])

        xn_pool = ctx.enter_context(tc.tile_pool(name="xn", bufs=CB))
        xn_sb = [xn_pool.tile([128, HW], BF16, tag="xn", name="xn") for _ in range(CB)]
        with tc.tile_pool(name="xf", bufs=3) as xf_pool:
            for half in range(2):
                cs = slice(half * (HW // 2), (half + 1) * (HW // 2))
                for j in range(CB):
                    xt = xf_pool.tile([128, HW // 2], F32, tag="xf", name="xf")
                    nc.sync.dma_start(xt[:, :], P["xf"][j * 128:(j + 1) * 128, cs])
                    nc.vector.tensor_scalar(
                        out=xn_sb[j][:, cs], in0=xt[:, :],
                        scalar1=scl_sb[:, j:j + 1], scalar2=bia_sb[:, j:j + 1],
                        op0=OP.mult, op1=OP.add)

        wq_sb = [load_const(f"wq{j}", [128, C], BF16, P["wqT"][j * 128:(j + 1) * 128, :]) for j in range(CB)]
        wk_sb = [load_const(f"wk{j}", [128, C], BF16, P["wkT"][j * 128:(j + 1) * 128, :]) for j in range(CB)]
        wv_sb = [load_const(f"wv{j}", [128, C], BF16, P["wvT"][j * 128:(j + 1) * 128, :]) for j in range(CB)]
        wp_sb = [load_const(f"wp{j}", [128, C], BF16, P["wpT"][j * 128:(j + 1) * 128, :]) for j in range(CB)]
        bq_sb = load_const("bq", [128, CB], F32, P["bq2d"][:, :])
        bpe_sb = load_const("bpe", [128, CB], F32, P["bpe2d"][:, :])
        onesf_sb = consts.tile([128, 1], F32, name="onesf")
        nc.vector.memset(onesf_sb[:, :], 1.0)
        onesrow_sb = consts.tile([1, 128], F32, name="onesrow")
        nc.vector.memset(onesrow_sb[:, :], 1.0)

        q_pool = ctx.enter_context(tc.tile_pool(name="q", bufs=CB))
        q_sb = [q_pool.tile([128, HALF], BF16, tag="q", name="q") for _ in range(CB)]
        k_pool = ctx.enter_context(tc.tile_pool(name="k", bufs=CB))
        k_sb = [k_pool.tile([128, HW], BF16, tag="k", name="k") for _ in range(CB)]
        v_pool = ctx.enter_context(tc.tile_pool(name="v", bufs=NKT))
        v_sb = [v_pool.tile([128, C], BF16, tag="v", name="v") for _ in range(NKT)]

        # psum pools: 4 + 3 + 1 = 8 banks
        ps_mm = ctx.enter_context(tc.tile_pool(name="ps_mm", bufs=4, space="PSUM"))
        ps_st = ctx.enter_context(tc.tile_pool(name="ps_st", bufs=3, space="PSUM"))
        ps_dn = ctx.enter_context(tc.tile_pool(name="ps_dn", bufs=1, space="PSUM"))

        p_pool = ctx.enter_context(tc.tile_pool(name="p", bufs=3))
        acc_pool = ctx.enter_context(tc.tile_pool(name="acc", bufs=2))
        dnr_pool = ctx.enter_context(tc.tile_pool(name="dnr", bufs=2))
        bc_pool = ctx.enter_context(tc.tile_pool(name="bc", bufs=2))
        atB_pool = ctx.enter_context(tc.tile_pool(name="atB", bufs=8))
        xr_pool = ctx.enter_context(tc.tile_pool(name="xr", bufs=3))
        ob_pool = ctx.enter_context(tc.tile_pool(name="ob", bufs=3))

        # ---------------- phase 1: q, k (channel-major), v (token-major) ----
        # Emit ALL work that only touches the first half of the frame
        # (q entirely + k/v first half) before anything needing the second
        # half: the second half's DMA+normalize is still in flight while
        # the PE chews through ~40us of first-half matmuls.
        def qk_group(w_sb, out_sb, j, t_, bias=None):
            ps = ps_mm.tile([128, 512], F32, tag="mm", name="mm")
            for ci in range(CB):
                nc.tensor.matmul(ps[:, :],
                                 lhsT=w_sb[ci][:, j * 128:(j + 1) * 128],
                                 rhs=xn_sb[ci][:, t_ * 512:(t_ + 1) * 512],
                                 start=(ci == 0), stop=(ci == CB - 1))
            dst = out_sb[j][:, t_ * 512:(t_ + 1) * 512]
            if bias is not None:
                nc.scalar.activation(dst, ps[:, :], AF.Identity, bias=bias)
            else:
                nc.scalar.copy(dst, ps[:, :])

        def v_group(m):
            ps = ps_mm.tile([128, 512], F32, tag="mm", name="mm")
            for ci in range(CB):
                nc.tensor.matmul(ps[:, :],
                                 lhsT=xn_sb[ci][:, m * 128:(m + 1) * 128],
                                 rhs=wv_sb[ci][:, :],
                                 start=(ci == 0), stop=(ci == CB - 1))
            nc.vector.tensor_copy(v_sb[m][:, :], ps[:, :])

        for j in range(CB):          # q covers exactly the first half
            for t_ in range(QG):
                qk_group(wq_sb, q_sb, j, t_, bias=bq_sb[:, j:j + 1])
        for j in range(CB):          # k, first half
            for t_ in range(4):
                qk_group(wk_sb, k_sb, j, t_)
        for m in range(NKT // 2):    # v, first half
            v_group(m)
        for j in range(CB):          # k, second half
            for t_ in range(4, 8):
                qk_group(wk_sb, k_sb, j, t_)
        for m in range(NKT // 2, NKT):
            v_group(m)

        # ---------------- phase 2: attention + proj per query group --------
        # proj of group g is emitted at the START of group g+1: its matmuls
        # are ready instantly (own psum pool, inputs done) and fill the PE
        # window where the next score matmuls wait on the denominator lag.
        def emit_proj(atB_sb, bc, q0):
            for cb in range(CB):
                pp = ps_dn.tile([128, 512], F32, tag="dn", name="pp")
                for j in range(CB):
                    nc.tensor.matmul(pp[:, :],
                                     lhsT=wp_sb[j][:, cb * 128:(cb + 1) * 128],
                                     rhs=atB_sb[j][:, :],
                                     start=(j == 0), stop=(j == CB - 1))
                xr = xr_pool.tile([128, 512], F32, tag="xr", name="xr")
                nc.scalar.dma_start(xr[:, :], P["xf"][cb * 128:(cb + 1) * 128, q0:q0 + 512])
                t1 = ob_pool.tile([128, 512], F32, tag="t1", name="t1")
                nc.vector.tensor_mul(t1[:, :], pp[:, :], bc[:, :])
                ob = ob_pool.tile([128, 512], F32, tag="ob", name="ob")
                nc.vector.scalar_tensor_tensor(ob[:, :], in0=t1[:, :],
                                               scalar=bpe_sb[:, cb:cb + 1],
                                               in1=xr[:, :],
                                               op0=OP.add, op1=OP.add)
                nc.sync.dma_start(P["out"][cb * 128:(cb + 1) * 128, q0:q0 + 512], ob[:, :])

        deferred = None
        for qg in range(QG):
            q0 = qg * 512
            pv = [ps_mm.tile([128, 512], F32, tag="mm", name="mm") for _ in range(CB)]
            if deferred is not None:
                emit_proj(*deferred)
                deferred = None
            acc = acc_pool.tile([128, 512], F32, tag="acc", name="acc")
            for m in range(NKT):
                st = ps_st.tile([128, 512], F32, tag="st", name="st")
                for j in range(CB):
                    nc.tensor.matmul(st[:, :],
                                     lhsT=k_sb[j][:, m * 128:(m + 1) * 128],
                                     rhs=q_sb[j][:, q0:q0 + 512],
                                     start=(j == 0), stop=(j == CB - 1))
                p = p_pool.tile([128, 512], BF16, tag="p", name="p")
                nc.scalar.activation(p[:, :], st[:, :], AF.Exp, scale=SCALE)
                if m == 0:
                    nc.vector.tensor_copy(acc[:, :], p[:, :])
                else:
                    nc.vector.tensor_add(acc[:, :], acc[:, :], p[:, :])
                for cb in range(CB):
                    # attention output channel-major: out[co, qt] += v^T p
                    nc.tensor.matmul(pv[cb][:, :],
                                     lhsT=v_sb[m][:, cb * 128:(cb + 1) * 128],
                                     rhs=p[:, :],
                                     start=(m == 0), stop=(m == NKT - 1))
            # copy UNNORMALIZED attention out of PSUM right away (frees the
            # pv banks for the next query group); the softmax denominator is
            # applied after the (linear) projection instead.
            atB_sb = []
            for cb in range(CB):
                atB = atB_pool.tile([128, 512], BF16, tag="atB", name="atB")
                nc.scalar.copy(atB[:, :], pv[cb][:, :])
                atB_sb.append(atB)
            # denominator: partition-reduce acc -> [1,512] -> 1/x -> rank-1
            # broadcast [128,512]; overlaps with the proj matmuls below
            dnr = ps_dn.tile([1, 512], F32, tag="dn", name="dnr")
            nc.tensor.matmul(dnr[:, :], lhsT=onesf_sb[:, :], rhs=acc[:, :],
                             start=True, stop=True)
            dnrec = dnr_pool.tile([1, 512], F32, tag="dnr", name="dnrec")
            nc.vector.reciprocal(dnrec[:, :], dnr[:, :])
            bcp = ps_dn.tile([128, 512], F32, tag="dn", name="bcp")
            nc.tensor.matmul(bcp[:, :], lhsT=onesrow_sb[:, :], rhs=dnrec[:, :],
                             start=True, stop=True)
            bc = bc_pool.tile([128, 512], F32, tag="bc", name="bc")
            nc.scalar.copy(bc[:, :], bcp[:, :])
            deferred = (atB_sb, bc, q0)
        emit_proj(*deferred)


def _build_main():
    nc = bacc.Bacc("TRN2", target_bir_lowering=False, debug=False,
                   num_devices=N_CORES)
    P = {}
    P["xf"] = nc.declare_dram_parameter("xf", [C, HW], F32, isOutput=False)
    for nm in ("wqT", "wkT", "wvT", "wpT"):
        P[nm] = nc.declare_dram_parameter(nm, [C, C], BF16, isOutput=False)
    for nm in ("bq2d", "bpe2d", "scl2d", "bia2d"):
        P[nm] = nc.declare_dram_parameter(nm, [128, CB], F32, isOutput=False)
    P["out"] = nc.declare_dram_parameter("out", [C, HALF], F32, isOutput=True)

    with tile.TileContext(nc) as tc:
        _body(tc, P)
    nc.finalize()
    return nc


def _get_ncs():
    if "nc" not in _CACHE:
        _CACHE["nc1"] = _build_stats()
        _CACHE["nc"] = _build_main()
    return _CACHE["nc1"], _CACHE["nc"]


def _frame_views(x):
    """Per-core rolled frame views: core i=(2f+h) sees frame f with its own
    half first."""
    views = []
    for i in range(N_CORES):
        f, h = divmod(i, 2)
        xfr = x[0, :, f].reshape(C, HW)
        if h == 1:
            xfr = np.concatenate([xfr[:, HALF:], xfr[:, :HALF]], axis=1)
        views.append(np.ascontiguousarray(xfr))
    return views


def _combine_stats(pstats_list, gamma, beta):
    """Host-side gather of kernel-1 partials -> per-channel scale/bias."""
    tot = np.zeros((128, 2 * CB), np.float64)
    for ps in pstats_list:
        tot += np.asarray(ps, np.float64)
    # column j holds channels [128j, 128j+128)
    s = tot[:, 0:CB].T.reshape(C)       # per-channel sum
    s2 = tot[:, CB:2 * CB].T.reshape(C)  # per-channel sumsq
    gs = s.reshape(G, C // G).sum(1)
    gs2 = s2.reshape(G, C // G).sum(1)
    meang = gs / NG_ELEMS
    varg = gs2 / NG_ELEMS - meang * meang
    rstd = 1.0 / np.sqrt(varg + EPS)
    chs = (np.asarray(gamma, np.float64) * np.repeat(rstd, C // G))
    chb = np.asarray(beta, np.float64) - np.repeat(meang, C // G) * chs
    def blk2d(v):
        return np.ascontiguousarray(v.astype(np.float32).reshape(CB, 128).T)
    return blk2d(chs), blk2d(chb)


def run_with_results(inputs, trace=False, **kw):
    bf16 = ml_dtypes.bfloat16
    f32 = np.float32
    x = np.asarray(inputs["x"], f32)
    gamma = np.asarray(inputs["gamma"], f32)
    beta = np.asarray(inputs["beta"], f32)
    wq, wk, wv, wp = [np.asarray(inputs[n], f32) for n in ("wq", "wk", "wv", "wp")]
    bq, bv, bp = [np.asarray(inputs[n], f32) for n in ("bq", "bv", "bp")]

    nc1, nc2 = _get_ncs()
    views = _frame_views(x)

    # ---- launch 1: partial GroupNorm stats over disjoint half-frames
    maps1 = [{"xh": views[i][:, :HALF]} for i in range(N_CORES)]
    maps1 = [{"xh": np.ascontiguousarray(m["xh"])} for m in maps1]
    res1 = run_bass_kernel_spmd(nc1, maps1, core_ids=list(range(N_CORES)),
                                trace=trace, **kw)
    scl2d, bia2d = _combine_stats([r["pstats"] for r in res1.results],
                                  gamma, beta)

    # ---- launch 2: the block itself
    def wT(w):
        return np.ascontiguousarray(w.T).astype(bf16)

    def blk2d(v):
        return np.ascontiguousarray(np.asarray(v, f32).reshape(CB, 128).T)

    shared = {
        "wqT": wT(wq), "wkT": wT(wk), "wvT": wT(wv), "wpT": wT(wp),
        "bq2d": blk2d(bq), "bpe2d": blk2d(bp + wp @ bv),
        "scl2d": scl2d, "bia2d": bia2d,
    }
    maps2 = [dict(shared, xf=views[i]) for i in range(N_CORES)]
    res2 = run_bass_kernel_spmd(nc2, maps2, core_ids=list(range(N_CORES)),
                                trace=trace, **kw)

    frames = []
    for f in range(T):
        a = np.asarray(res2.results[2 * f]["out"], dtype=np.float32)
        b = np.asarray(res2.results[2 * f + 1]["out"], dtype=np.float32)
        frames.append(np.concatenate([a, b], axis=1))
    out = np.stack(frames, axis=1)           # (C, T, HW)
    out = np.ascontiguousarray(out.reshape(1, C, T, 64, 64))
    return out, (res1, res2)


def kernel(**inputs):
    out, _ = run_with_results(inputs)
    return out



# revision 7
# speedup vs baseline: 1.5900x; 1.5900x over previous
"""GroupNorm + per-frame spatial attention block on 8 TRN2 NeuronCores.

Problem shape: x (1, 512, 4, 64, 64) f32.
  y   = GroupNorm32(x) (stats over (c/32, t, h, w) -> global over all frames)
  tok = y as (t, hw=4096, c=512)
  q,k,v = tok @ w{q,k,v}.T + b ; per-frame softmax(q k^T / sqrt(c)) v
  out = attn @ wp.T + bp ; return x + out

Sharding: core i handles frame f=i//2, query-half h=i%2 (2048 queries).
Each core redundantly computes K/V for its whole frame (cheaper than an
intra-pair all-gather).

Two launches (a fleet-wide collective barrier costs ~65us of latency, so
the tiny GroupNorm stats reduction is done as its own collective-free
kernel; the host combines the 8x[128,16] partial sums while "gathering"):
  kernel 1: per-core partial sum/sumsq over its disjoint half-frame,
            8 chunks spread over 4 DMA queues.
  host:     combine partials -> per-channel scale/bias (512 numbers).
  kernel 2: normalize + qkv + attention + proj + residual.

All O(n*c^2)/O(n^2*c) matmuls except the projection run in fp8e4 with
perf_mode=DoubleRow (measured 1.87x over bf16 for these shapes: a DR
matmul contracts 256 partitions in the same ~223ns a bf16 matmul needs
for 128).  fp8 operand layout: contraction-pair blocks side by side in
the free dim, sliced as 3D APs [128, 2, n] per tile_matmul's pattern.

Scaling scheme (exact, folded into existing instructions):
  - weights wq/wk/wv are scaled x16 on host before the e4m3 cast (raw
    values ~N(0, 1/512) sit below e4m3's subnormal floor).
  - q,k,v stay x16 in SBUF (|16q|<~91 << 240): the x256 on scores is
    folded into the exp scale, v's x16 into the reciprocal broadcast.
  - p = exp(s - 2): shift keeps p <= ~70 inside e4m3 range (exp(s) can
    reach 510 > 240 -> Inf).  The shift cancels in softmax exactly.
  - bk drops out of softmax; bv is folded into the proj bias on host
    (bp_eff = bp + wp @ bv); softmax denominator applied post-proj.

Measured on 8xTRN2 (NTFF profile): see test.py output.
"""

import numpy as np
import ml_dtypes

import concourse.bass as bass
import concourse.bacc as bacc
import concourse.tile as tile
from concourse import mybir
from concourse.bass_utils import run_bass_kernel_spmd

C = 512
T = 4
HW = 64 * 64          # tokens per frame
HALF = HW // 2        # local queries per core
G = 32                # groups
N_CORES = 8
EPS = 1e-6
NG_ELEMS = (C // G) * T * HW   # elements per group in the full tensor
CB = C // 128         # 4 channel blocks
QG = HALF // 512      # 4 query groups of 512
NKT = HW // 128       # 32 key chunks of 128
NPAIR = NKT // 2      # 16 key-pair chunks of 256
WS = 16.0             # fp8 weight scale
SCALE = float(C) ** -0.5 / (WS * WS)   # exp input scale (q,k carry x16)
ESHIFT = -2.0         # exp(s - 2): keeps p within e4m3 range

BF16 = mybir.dt.bfloat16
F32 = mybir.dt.float32
F8 = mybir.dt.float8e4
DR = mybir.MatmulPerfMode.DoubleRow
AX = mybir.AxisListType
AF = mybir.ActivationFunctionType
OP = mybir.AluOpType

_CACHE = {}


# ---------------------------------------------------------------- kernel 1
def _build_stats():
    nc = bacc.Bacc("TRN2", target_bir_lowering=False, debug=False,
                   num_devices=N_CORES)
    xh = nc.declare_dram_parameter("xh", [C, HALF], F32, isOutput=False)
    pstats = nc.declare_dram_parameter("pstats", [128, 16], F32,
                                       isOutput=True)
    HH = HALF // 2
    with tile.TileContext(nc) as tc:
        queues = [nc.sync, nc.gpsimd]  # DMA-capable engines (scalar does squares)
        with tc.tile_pool(name="xt", bufs=4) as xt_pool, \
             tc.tile_pool(name="scr", bufs=2) as scr_pool, \
             tc.tile_pool(name="st", bufs=1) as st_pool:
            # sums on DVE, sums-of-squares on ACT: the two run in parallel
            stats_sb = st_pool.tile([128, 16], F32, name="stats")
            stats2_sb = st_pool.tile([128, 8], F32, name="stats2")
            for idx in range(8):
                j, h = divmod(idx, 2)
                xt = xt_pool.tile([128, HH], F32, tag="xt", name="xt")
                queues[idx % 2].dma_start(
                    xt[:, :], xh[j * 128:(j + 1) * 128, h * HH:(h + 1) * HH])
                nc.vector.reduce_sum(stats_sb[:, idx:idx + 1], xt[:, :], axis=AX.X)
                scr = scr_pool.tile([128, HH], F32, tag="scr", name="scr")
                nc.scalar.activation(scr[:, :], xt[:, :], AF.Square,
                                     accum_out=stats2_sb[:, idx:idx + 1])
            nc.vector.tensor_copy(stats_sb[:, 8:16], stats2_sb[:, :])
            nc.sync.dma_start(pstats[:, :], stats_sb[:, :])
    nc.finalize()
    return nc


# ---------------------------------------------------------------- kernel 2
def _body(tc, P):
    from contextlib import ExitStack

    nc = tc.nc
    with ExitStack() as ctx:
        consts = ctx.enter_context(tc.tile_pool(name="consts", bufs=1))

        def load_const(name, shape, dtype, src, engine=None):
            t_ = consts.tile(shape, dtype, name=name)
            (engine or nc.scalar).dma_start(t_[...], src)
            return t_

        # normalize scale/bias first (first normalize waits on these),
        # then the fp8 qkv weights; wp/proj consts follow later.
        scl_sb = load_const("scl", [128, CB], F32, P["scl2d"][:, :])
        bia_sb = load_const("bia", [128, CB], F32, P["bia2d"][:, :])
        wq_sb = load_const("wq8", [128, CB, C], F8, P["wq8"][:, :])
        wk_sb = load_const("wk8", [128, CB, C], F8, P["wk8"][:, :])
        wv_sb = load_const("wv8", [128, CB, C], F8, P["wv8"][:, :])
        bq_sb = load_const("bq", [128, CB], F32, P["bq2d"][:, :])

        xn_pool = ctx.enter_context(tc.tile_pool(name="xn", bufs=1))
        xnA = xn_pool.tile([128, 2, HW], F8, name="xnA")
        xnB = xn_pool.tile([128, 2, HW], F8, name="xnB")
        q_pool = ctx.enter_context(tc.tile_pool(name="q", bufs=1))
        qA = q_pool.tile([128, 2, HALF], F8, name="qA")
        qB = q_pool.tile([128, 2, HALF], F8, name="qB")
        k_pool = ctx.enter_context(tc.tile_pool(name="k", bufs=1))
        kA = k_pool.tile([128, 2, HW], F8, name="kA")
        kB = k_pool.tile([128, 2, HW], F8, name="kB")
        v_pool = ctx.enter_context(tc.tile_pool(name="v", bufs=1))
        v_all = v_pool.tile([128, NKT, C], F8, name="v_all")

        # psum pools: 4 + 3 + 1 = 8 banks
        ps_mm = ctx.enter_context(tc.tile_pool(name="ps_mm", bufs=4, space="PSUM"))
        ps_st = ctx.enter_context(tc.tile_pool(name="ps_st", bufs=3, space="PSUM"))
        ps_dn = ctx.enter_context(tc.tile_pool(name="ps_dn", bufs=1, space="PSUM"))

        p_pool = ctx.enter_context(tc.tile_pool(name="p", bufs=3))
        acc_pool = ctx.enter_context(tc.tile_pool(name="acc", bufs=2))
        dnr_pool = ctx.enter_context(tc.tile_pool(name="dnr", bufs=2))
        bc_pool = ctx.enter_context(tc.tile_pool(name="bc", bufs=2))
        atB_pool = ctx.enter_context(tc.tile_pool(name="atB", bufs=8))
        xr_pool = ctx.enter_context(tc.tile_pool(name="xr", bufs=3))
        ob_pool = ctx.enter_context(tc.tile_pool(name="ob", bufs=3))
        xf_pool = ctx.enter_context(tc.tile_pool(name="xf", bufs=6))

        dmaq = [nc.sync, nc.gpsimd]

        # ---------------- phase 1: per 512-token group: load+normalize,
        # then q (first half only), k, v.  PE executes in emission order,
        # so DMA/normalize of group t+1 overlaps the matmuls of group t.
        def emit_consts_late():
            wp_sb = [load_const(f"wp{j}", [128, C], BF16,
                                P["wpT"][j * 128:(j + 1) * 128, :])
                     for j in range(CB)]
            bpe_sb = load_const("bpe", [128, CB], F32, P["bpe2d"][:, :])
            onesf_sb = consts.tile([128, 1], F32, name="onesf")
            nc.vector.memset(onesf_sb[:, :], 1.0)
            onesrow_sb = consts.tile([1, 128], F32, name="onesrow")
            # 1/WS folds v's x16 into the softmax normalization
            nc.vector.memset(onesrow_sb[:, :], 1.0 / WS)
            eshift_sb = consts.tile([128, 1], F32, name="eshift")
            nc.vector.memset(eshift_sb[:, :], ESHIFT)
            return wp_sb, bpe_sb, onesf_sb, onesrow_sb, eshift_sb

        late = None
        for t_ in range(2 * QG):
            cs = slice(t_ * 512, (t_ + 1) * 512)
            for cb in range(CB):
                xt = xf_pool.tile([128, 512], F32, tag="xf", name="xf")
                dmaq[(t_ * CB + cb) % 2].dma_start(
                    xt[:, :], P["xf"][cb * 128:(cb + 1) * 128, cs])
                dst = (xnA, xnB)[cb // 2]
                nc.vector.tensor_scalar(
                    out=dst[:, cb % 2:cb % 2 + 1, cs], in0=xt[:, :],
                    scalar1=scl_sb[:, cb:cb + 1], scalar2=bia_sb[:, cb:cb + 1],
                    op0=OP.mult, op1=OP.add)
            if t_ == 0:
                late = emit_consts_late()
            if t_ < QG:          # q: only the core's own query half
                for j in range(CB):
                    ps = ps_mm.tile([128, 512], F32, tag="mm", name="mm")
                    nc.tensor.matmul(ps[:, :], lhsT=wq_sb[:, 0:2, j * 128:(j + 1) * 128],
                                     rhs=xnA[:, :, cs], start=True, stop=False,
                                     perf_mode=DR)
                    nc.tensor.matmul(ps[:, :], lhsT=wq_sb[:, 2:4, j * 128:(j + 1) * 128],
                                     rhs=xnB[:, :, cs], start=False, stop=True,
                                     perf_mode=DR)
                    dst = (qA, qB)[j // 2]
                    nc.scalar.activation(dst[:, j % 2:j % 2 + 1, cs], ps[:, :],
                                         AF.Identity, bias=bq_sb[:, j:j + 1])
            for j in range(CB):  # k (channel-major, whole frame)
                ps = ps_mm.tile([128, 512], F32, tag="mm", name="mm")
                nc.tensor.matmul(ps[:, :], lhsT=wk_sb[:, 0:2, j * 128:(j + 1) * 128],
                                 rhs=xnA[:, :, cs], start=True, stop=False,
                                 perf_mode=DR)
                nc.tensor.matmul(ps[:, :], lhsT=wk_sb[:, 2:4, j * 128:(j + 1) * 128],
                                 rhs=xnB[:, :, cs], start=False, stop=True,
                                 perf_mode=DR)
                dst = (kA, kB)[j // 2]
                nc.scalar.copy(dst[:, j % 2:j % 2 + 1, cs], ps[:, :])
            for mi in range(4):  # v (token-major, whole frame)
                m = t_ * 4 + mi
                ms = slice(m * 128, (m + 1) * 128)
                ps = ps_mm.tile([128, 512], F32, tag="mm", name="mm")
                nc.tensor.matmul(ps[:, :], lhsT=xnA[:, :, ms], rhs=wv_sb[:, 0:2, :],
                                 start=True, stop=False, perf_mode=DR)
                nc.tensor.matmul(ps[:, :], lhsT=xnB[:, :, ms], rhs=wv_sb[:, 2:4, :],
                                 start=False, stop=True, perf_mode=DR)
                nc.vector.tensor_copy(v_all[:, m:m + 1, :], ps[:, :])

        wp_sb, bpe_sb, onesf_sb, onesrow_sb, eshift_sb = late

        # ---------------- phase 2: attention + proj per query group --------
        # proj of group g is emitted at the START of group g+1: its matmuls
        # are ready instantly (own psum pool, inputs done) and fill the PE
        # window where the next score matmuls wait on the denominator lag.
        def emit_proj(atB_sb, bc, q0):
            for cb in range(CB):
                pp = ps_dn.tile([128, 512], F32, tag="dn", name="pp")
                for j in range(CB):
                    nc.tensor.matmul(pp[:, :],
                                     lhsT=wp_sb[j][:, cb * 128:(cb + 1) * 128],
                                     rhs=atB_sb[j][:, :],
                                     start=(j == 0), stop=(j == CB - 1))
                xr = xr_pool.tile([128, 512], F32, tag="xr", name="xr")
                nc.scalar.dma_start(xr[:, :], P["xf"][cb * 128:(cb + 1) * 128, q0:q0 + 512])
                t1 = ob_pool.tile([128, 512], F32, tag="t1", name="t1")
                nc.vector.tensor_mul(t1[:, :], pp[:, :], bc[:, :])
                ob = ob_pool.tile([128, 512], F32, tag="ob", name="ob")
                nc.vector.scalar_tensor_tensor(ob[:, :], in0=t1[:, :],
                                               scalar=bpe_sb[:, cb:cb + 1],
                                               in1=xr[:, :],
                                               op0=OP.add, op1=OP.add)
                nc.sync.dma_start(P["out"][cb * 128:(cb + 1) * 128, q0:q0 + 512], ob[:, :])

        deferred = None
        for qg in range(QG):
            q0 = qg * 512
            qs = slice(q0, q0 + 512)
            pv = [ps_mm.tile([128, 512], F32, tag="mm", name="mm") for _ in range(CB)]
            if deferred is not None:
                emit_proj(*deferred)
                deferred = None
            acc = acc_pool.tile([128, 512], F32, tag="acc", name="acc")
            for r in range(NPAIR):
                p2 = p_pool.tile([128, 2, 512], F8, tag="p", name="p")
                for half in range(2):
                    m = 2 * r + half
                    ms = slice(m * 128, (m + 1) * 128)
                    st = ps_st.tile([128, 512], F32, tag="st", name="st")
                    nc.tensor.matmul(st[:, :], lhsT=kA[:, :, ms], rhs=qA[:, :, qs],
                                     start=True, stop=False, perf_mode=DR)
                    nc.tensor.matmul(st[:, :], lhsT=kB[:, :, ms], rhs=qB[:, :, qs],
                                     start=False, stop=True, perf_mode=DR)
                    nc.scalar.activation(p2[:, half:half + 1, :], st[:, :],
                                         AF.Exp, scale=SCALE, bias=eshift_sb[:, :])
                    if m == 0:
                        nc.vector.tensor_copy(acc[:, :], p2[:, 0:1, :])
                    else:
                        nc.vector.tensor_add(acc[:, :], acc[:, :], p2[:, half:half + 1, :])
                for cb in range(CB):
                    # attention output channel-major: out[co, qt] += v^T p
                    nc.tensor.matmul(pv[cb][:, :],
                                     lhsT=v_all[:, 2 * r:2 * r + 2, cb * 128:(cb + 1) * 128],
                                     rhs=p2[:, :, :],
                                     start=(r == 0), stop=(r == NPAIR - 1),
                                     perf_mode=DR)
            # copy UNNORMALIZED attention out of PSUM right away (frees the
            # pv banks for the next query group); the softmax denominator is
            # applied after the (linear) projection instead.
            atB_sb = []
            for cb in range(CB):
                atB = atB_pool.tile([128, 512], BF16, tag="atB", name="atB")
                nc.scalar.copy(atB[:, :], pv[cb][:, :])
                atB_sb.append(atB)
            # denominator: partition-reduce acc -> [1,512] -> 1/x -> rank-1
            # broadcast [128,512]; overlaps with the proj matmuls below
            dnr = ps_dn.tile([1, 512], F32, tag="dn", name="dnr")
            nc.tensor.matmul(dnr[:, :], lhsT=onesf_sb[:, :], rhs=acc[:, :],
                             start=True, stop=True)
            dnrec = dnr_pool.tile([1, 512], F32, tag="dnr", name="dnrec")
            nc.vector.reciprocal(dnrec[:, :], dnr[:, :])
            bcp = ps_dn.tile([128, 512], F32, tag="dn", name="bcp")
            nc.tensor.matmul(bcp[:, :], lhsT=onesrow_sb[:, :], rhs=dnrec[:, :],
                             start=True, stop=True)
            bc = bc_pool.tile([128, 512], F32, tag="bc", name="bc")
            nc.scalar.copy(bc[:, :], bcp[:, :])
            deferred = (atB_sb, bc, q0)
        emit_proj(*deferred)


def _build_main():
    nc = bacc.Bacc("TRN2", target_bir_lowering=False, debug=False,
                   num_devices=N_CORES)
    P = {}
    P["xf"] = nc.declare_dram_parameter("xf", [C, HW], F32, isOutput=False)
    for nm in ("wq8", "wk8", "wv8"):
        P[nm] = nc.declare_dram_parameter(nm, [128, CB * C], F8, isOutput=False)
    P["wpT"] = nc.declare_dram_parameter("wpT", [C, C], BF16, isOutput=False)
    for nm in ("bq2d", "bpe2d", "scl2d", "bia2d"):
        P[nm] = nc.declare_dram_parameter(nm, [128, CB], F32, isOutput=False)
    P["out"] = nc.declare_dram_parameter("out", [C, HALF], F32, isOutput=True)

    with tile.TileContext(nc) as tc:
        _body(tc, P)
    nc.finalize()
    return nc


def _get_ncs():
    if "nc" not in _CACHE:
        _CACHE["nc1"] = _build_stats()
        _CACHE["nc"] = _build_main()
    return _CACHE["nc1"], _CACHE["nc"]


def _frame_views(x):
    """Per-core rolled frame views: core i=(2f+h) sees frame f with its own
    half first."""
    views = []
    for i in range(N_CORES):
        f, h = divmod(i, 2)
        xfr = x[0, :, f].reshape(C, HW)
        if h == 1:
            xfr = np.concatenate([xfr[:, HALF:], xfr[:, :HALF]], axis=1)
        views.append(np.ascontiguousarray(xfr))
    return views


def _combine_stats(pstats_list, gamma, beta):
    """Host-side gather of kernel-1 partials -> per-channel scale/bias."""
    tot = np.zeros((128, 16), np.float64)
    for ps in pstats_list:
        tot += np.asarray(ps, np.float64)
    # col 2j+h holds channels [128j, 128j+128), half h
    s = tot[:, 0:8].reshape(128, CB, 2).sum(-1).T.reshape(C)
    s2 = tot[:, 8:16].reshape(128, CB, 2).sum(-1).T.reshape(C)
    gs = s.reshape(G, C // G).sum(1)
    gs2 = s2.reshape(G, C // G).sum(1)
    meang = gs / NG_ELEMS
    varg = gs2 / NG_ELEMS - meang * meang
    rstd = 1.0 / np.sqrt(varg + EPS)
    chs = (np.asarray(gamma, np.float64) * np.repeat(rstd, C // G))
    chb = np.asarray(beta, np.float64) - np.repeat(meang, C // G) * chs
    def blk2d(v):
        return np.ascontiguousarray(v.astype(np.float32).reshape(CB, 128).T)
    return blk2d(chs), blk2d(chb)


def run_with_results(inputs, trace=False, **kw):
    f8 = ml_dtypes.float8_e4m3
    bf16 = ml_dtypes.bfloat16
    f32 = np.float32
    x = np.asarray(inputs["x"], f32)
    gamma = np.asarray(inputs["gamma"], f32)
    beta = np.asarray(inputs["beta"], f32)
    wq, wk, wv, wp = [np.asarray(inputs[n], f32) for n in ("wq", "wk", "wv", "wp")]
    bq, bv, bp = [np.asarray(inputs[n], f32) for n in ("bq", "bv", "bp")]

    nc1, nc2 = _get_ncs()
    views = _frame_views(x)

    # ---- launch 1: partial GroupNorm stats over disjoint half-frames
    maps1 = [{"xh": np.ascontiguousarray(views[i][:, :HALF])}
             for i in range(N_CORES)]
    res1 = run_bass_kernel_spmd(nc1, maps1, core_ids=list(range(N_CORES)),
                                trace=trace, **kw)
    scl2d, bia2d = _combine_stats([r["pstats"] for r in res1.results],
                                  gamma, beta)

    # ---- launch 2: the block itself
    def w8(w):
        # [128, cb, c_out] fp8: w.T blocked by c_in, x16 against the
        # subnormal floor (folded back out via SCALE / onesrow)
        wt = (w.T * WS).reshape(CB, 128, C).transpose(1, 0, 2)
        return np.ascontiguousarray(wt.astype(f8).reshape(128, CB * C))

    def blk2d(v):
        return np.ascontiguousarray(np.asarray(v, f32).reshape(CB, 128).T)

    shared = {
        "wq8": w8(wq), "wk8": w8(wk), "wv8": w8(wv),
        "wpT": np.ascontiguousarray(wp.T).astype(bf16),
        "bq2d": blk2d(bq * WS), "bpe2d": blk2d(bp + wp @ bv),
        "scl2d": scl2d, "bia2d": bia2d,
    }
    maps2 = [dict(shared, xf=views[i]) for i in range(N_CORES)]
    res2 = run_bass_kernel_spmd(nc2, maps2, core_ids=list(range(N_CORES)),
                                trace=trace, **kw)

    frames = []
    for f in range(T):
        a = np.asarray(res2.results[2 * f]["out"], dtype=np.float32)
        b = np.asarray(res2.results[2 * f + 1]["out"], dtype=np.float32)
        frames.append(np.concatenate([a, b], axis=1))
    out = np.stack(frames, axis=1)           # (C, T, HW)
    out = np.ascontiguousarray(out.reshape(1, C, T, 64, 64))
    return out, (res1, res2)


def kernel(**inputs):
    out, _ = run_with_results(inputs)
    return out
